# revision 1
# baseline (speedup 1.0000x reference)
"""TRN2 Bass kernel: transformer Block (LN->MHA->2x residual->LN->MLP) for
B=32,N=512,C=768,H=12. Data-parallel over batch across 8 NeuronCores (4
items/core). All matmuls run on the PE in float32r (full-rate fp32 mode,
1 cyc/row at N>=256).

Per-core program:
  prologue: PE-transpose qkv/proj weights into [c-on-partition] layout
  phase 1 (per batch item): LN1 -> h0 -> PE-transpose -> qkT/v matmuls ->
    per-head scoresT = kT.T@qT -> exp (no max-sub; scores are N(0,1)-scale) ->
    [v|1]-augmented AV matmul (oT + softmax denominators in one pass) ->
    normalize via reciprocal + PE-broadcast -> proj -> x2=2*(proj+proj_b) ->
    spill x2 to DRAM
  phase 2a (t-chunks of 512): LN2 -> h2T -> fc1 -> gelu -> spill fc1outT
  phase 2b (t-chunks of 512): fc2 -> + x2 + fc2_b -> out
"""
import json
import os
import tempfile

import numpy as np
from contextlib import ExitStack

import concourse.bass as bass
import concourse.tile as tile
import concourse.bacc as bacc
from concourse import mybir
from concourse.bass_utils import run_bass_kernel_spmd
from concourse.masks import make_identity

F32 = mybir.dt.float32
F32R = mybir.dt.float32r
AF = mybir.ActivationFunctionType
ALU = mybir.AluOpType

B, N, C = 32, 512, 768
H, D = 12, 64
HID = 4 * C
EPS = 1e-5
NCORES = 8
BPC = B // NCORES            # batch items per core
T = BPC * N                  # tokens per core
CK = C // 128                # 6 contraction chunks over C
FQK = (2 * C) // 128         # 12 feature tiles for q+k
JH = HID // 128              # 24 hidden feature tiles
NT = N // 128                # 4 token tiles per item
SCALE = D ** -0.5
TC2 = 512                    # phase-2 token chunk


def _bc(ap, p=128):
    """Broadcast a 1-D DRAM AP across p partitions (stride-0 partition dim)."""
    return bass.AP(tensor=ap.tensor, offset=ap.offset, ap=[[0, p]] + list(ap.ap))


def _emit(tc, io, ctx):
    nc = tc.nc

    consts = ctx.enter_context(tc.tile_pool(name="consts", bufs=1))
    wbig = ctx.enter_context(tc.tile_pool(name="wbig", bufs=1))
    small = ctx.enter_context(tc.tile_pool(name="small", bufs=4))
    xio = ctx.enter_context(tc.tile_pool(name="xio", bufs=2))
    ps1 = ctx.enter_context(tc.tile_pool(name="ps1", bufs=4, space="PSUM"))
    ps2 = ctx.enter_context(tc.tile_pool(name="ps2", bufs=2, space="PSUM"))
    dram = ctx.enter_context(tc.tile_pool(name="dram", bufs=1, space="DRAM"))

    # ---------------- constants ----------------
    ident32 = consts.tile([128, 128], F32)
    make_identity(nc, ident32)
    identr = consts.tile([128, 128], F32R)
    nc.vector.tensor_copy(out=identr, in_=ident32)
    onesf2 = consts.tile([128, 64], F32)
    nc.vector.memset(onesf2, 1.0)
    onesr = consts.tile([128, 64], F32R)
    nc.vector.tensor_copy(out=onesr, in_=onesf2)
    onecol = consts.tile([128, NT * H], F32)
    nc.vector.memset(onecol, 1.0)
    epst = consts.tile([128, 1], F32)
    nc.vector.memset(epst, EPS)

    ln1w_bc = consts.tile([128, C], F32)
    nc.sync.dma_start(out=ln1w_bc, in_=_bc(io["ln1_w"]))
    ln1b_bc = consts.tile([128, C], F32)
    nc.sync.dma_start(out=ln1b_bc, in_=_bc(io["ln1_b"]))
    ln2w_bc = consts.tile([128, C], F32)
    nc.sync.dma_start(out=ln2w_bc, in_=_bc(io["ln2_w"]))
    ln2b_bc = consts.tile([128, C], F32)
    nc.sync.dma_start(out=ln2b_bc, in_=_bc(io["ln2_b"]))
    pb2_bc = consts.tile([128, C], F32)
    nc.sync.dma_start(out=pb2_bc, in_=_bc(io["proj_b"]))
    nc.scalar.mul(out=pb2_bc, in_=pb2_bc, mul=2.0)
    fc2b_bc = consts.tile([128, C], F32)
    nc.sync.dma_start(out=fc2b_bc, in_=_bc(io["fc2_b"]))
    fc1b_t = consts.tile([128, JH], F32)
    nc.sync.dma_start(out=fc1b_t, in_=io["fc1_b"].rearrange("(j p) -> p j", p=128))

    # DRAM scratch
    x2d = dram.tile([T, C], F32)
    f1d = dram.tile([JH, 128, T], F32R)

    # ---------------- weight transposition helper ----------------
    evac_ctr = [0]

    def load_wT(w_ap, nrows, ncols, dst, stg):
        """w [nrows, ncols] row-major DRAM -> dst [128, ncols//128, nrows] F32R."""
        nj, nk = nrows // 128, ncols // 128
        wr = w_ap.rearrange("(j p) c -> p j c", p=128)
        for j in range(nj):
            for c0 in range(0, nk, 6):
                cn = min(6, nk - c0)
                piece = stg.tile([128, 768], F32, tag="wstage", name="piece")
                nc.sync.dma_start(out=piece[:, 0:cn * 128],
                                  in_=wr[:, j, c0 * 128:(c0 + cn) * 128])
                for k in range(cn):
                    tp = ps1.tile([128, 128], F32, tag="s1", name="tp")
                    nc.tensor.transpose(tp[:], piece[:, k * 128:(k + 1) * 128],
                                        ident32[:])
                    if evac_ctr[0] % 2 == 0:
                        nc.vector.tensor_copy(
                            out=dst[:, c0 + k, j * 128:(j + 1) * 128], in_=tp[:])
                    else:
                        nc.scalar.copy(
                            out=dst[:, c0 + k, j * 128:(j + 1) * 128], in_=tp[:])
                    evac_ctr[0] += 1

    def layer_norm(x_t, w_bcast, b_bcast, pool):
        """x_t [128, C] f32 -> returns h [128, C] F32R = LN(x)*w + b."""
        st = small.tile([128, 3, nc.vector.BN_STATS_DIM], F32, tag="bnst",
                        name="st")
        for i in range(3):
            nc.vector.bn_stats(out=st[:, i, :], in_=x_t[:, 256 * i:256 * (i + 1)])
        mv = small.tile([128, nc.vector.BN_AGGR_DIM], F32, tag="mv", name="mv")
        nc.vector.bn_aggr(out=mv, in_=st)
        rstd = small.tile([128, 1], F32, tag="rstd", name="rstd")
        nc.scalar.activation(out=rstd, in_=mv[:, 1:2], func=AF.Sqrt, bias=epst)
        nc.vector.reciprocal(out=rstd, in_=rstd)
        ht = pool.tile([128, C], F32, tag="lnt", bufs=1, name="ht")
        nc.vector.tensor_scalar(out=ht, in0=x_t, scalar1=mv[:, 0:1],
                                scalar2=rstd, op0=ALU.subtract, op1=ALU.mult)
        nc.vector.tensor_mul(out=ht, in0=ht, in1=w_bcast)
        h = pool.tile([128, C], F32R, tag="h0", bufs=1, name="h")
        nc.vector.tensor_add(out=h, in0=ht, in1=b_bcast)
        return h

    def transpose_to(h, dstT, tt):
        """h [128, C] F32R -> dstT[:, k, tt*128:(tt+1)*128] for k in CK."""
        for k in range(CK):
            tp = ps2.tile([128, 128], F32R, tag="s2", name="tp")
            nc.tensor.transpose(tp[:], h[:, k * 128:(k + 1) * 128], identr[:])
            if k % 2 == 0:
                nc.vector.tensor_copy(
                    out=dstT[:, k, tt * 128:(tt + 1) * 128], in_=tp[:])
            else:
                nc.scalar.copy(
                    out=dstT[:, k, tt * 128:(tt + 1) * 128], in_=tp[:])

    # ================= stage A: weights + phase 1 =================
    with tc.tile_pool(name="wstage_a", bufs=2) as wstage_a, \
         tc.tile_pool(name="wp", bufs=1) as wp_pool, \
         tc.tile_pool(name="p1", bufs=1) as p1:

        wqkvT = wbig.tile([128, CK, 3 * C], F32R, tag="w")
        load_wT(io["qkv_w"], 3 * C, C, wqkvT, wstage_a)
        wpT = wp_pool.tile([128, CK, C], F32R)
        load_wT(io["proj_w"], C, C, wpT, wstage_a)

        for b in range(BPC):
            t0 = b * N
            h0T = p1.tile([128, CK, N], F32R, tag="h0T", name="h0T")
            for tt in range(NT):
                x_t = xio.tile([128, C], F32, tag="xio", name="x_t")
                nc.sync.dma_start(
                    out=x_t, in_=io["x"][t0 + tt * 128:t0 + (tt + 1) * 128, :])
                h0 = layer_norm(x_t, ln1w_bc, ln1b_bc, p1)
                transpose_to(h0, h0T, tt)

            # qkT: feature tile j holds heads 2j / 2j+1 stacked on partitions
            qk_sb = p1.tile([128, FQK, N], F32R, tag="qk", name="qk_sb")
            for j in range(FQK):
                qp = ps1.tile([128, N], F32, tag="s1", name="qp")
                for k in range(CK):
                    nc.tensor.matmul(qp[:], wqkvT[:, k, j * 128:(j + 1) * 128],
                                     h0T[:, k, :], start=(k == 0),
                                     stop=(k == CK - 1))
                nc.scalar.copy(out=qk_sb[:, j, :], in_=qp[:])

            # v (tokens on partitions) with ones column at d=D
            v_sb = p1.tile([128, NT, H, D + 1], F32R, tag="v", name="v_sb")
            nc.vector.tensor_copy(
                out=v_sb[:, :, :, D:D + 1],
                in_=onecol.rearrange("p (a b c) -> p a b c", a=NT, b=H))
            for tt in range(NT):
                vp = ps2.tile([128, C], F32, tag="s2", name="vp")
                for k in range(CK):
                    for half, n0, nn in ((0, 0, 512), (1, 512, 256)):
                        nc.tensor.matmul(vp[:, n0:n0 + nn],
                                         h0T[:, k, tt * 128:(tt + 1) * 128],
                                         wqkvT[:, k, 2 * C + n0:2 * C + n0 + nn],
                                         start=(k == 0), stop=(k == CK - 1))
                nc.vector.tensor_copy(out=v_sb[:, tt, :, 0:D],
                                      in_=vp.rearrange("p (h d) -> p h d", h=H))

            # attention; oT: head h -> chunk h//2, partitions 64*(h%2)
            oT = p1.tile([128, CK, N], F32R, tag="oT", name="oT")
            for q4 in range(H // 4):
                srow = p1.tile([128, N], F32, tag="srow", bufs=2, name="srow")
                nc.vector.memset(srow, 1.0)
                orws = []
                for pi in range(2):
                    hp = 2 * q4 + pi
                    kj = FQK // 2 + hp
                    orw = p1.tile([128, N], F32, tag="orw", bufs=2, name="orw")
                    for sub in range(2):
                        h = 2 * hp + sub
                        p0 = 64 * sub
                        r = 32 * (h % 4)
                        av = ps1.tile([D + 1, N], F32, tag="s1", name="av")
                        for c in range(NT):
                            sc = ps1.tile([128, N], F32, tag="s1", name="sc")
                            nc.tensor.matmul(
                                sc[:],
                                qk_sb[p0:p0 + D, kj, c * 128:(c + 1) * 128],
                                qk_sb[p0:p0 + D, hp, :])
                            ex = p1.tile([128, N], F32R, tag="e5", bufs=2,
                                         name="ex")
                            nc.scalar.activation(out=ex, in_=sc[:], func=AF.Exp,
                                                 scale=SCALE)
                            nc.tensor.matmul(av[:], v_sb[:, c, h, :], ex[:],
                                             start=(c == 0), stop=(c == NT - 1))
                        # gather sums at 32-aligned rows; stash o rows
                        # (on DVE: ScalarE's exp gates the AV critical path)
                        nc.vector.tensor_copy(out=srow[r:r + 1, :],
                                              in_=av[D:D + 1, :])
                        nc.vector.tensor_copy(out=orw[p0:p0 + D, :],
                                              in_=av[0:D, :])
                    orws.append(orw)
                # one batched reciprocal for 4 heads (DVE div is 8 cyc/elem)
                rec4 = p1.tile([128, N], F32R, tag="srow", bufs=2, name="rec4")
                with nc.allow_low_precision(reason="softmax denom recip"):
                    nc.vector.reciprocal(out=rec4[0:97, 0:N // 2],
                                         in_=srow[0:97, 0:N // 2])
                    nc.vector.reciprocal(out=rec4[0:97, N // 2:N],
                                         in_=srow[0:97, N // 2:N])
                for pi in range(2):
                    hp = 2 * q4 + pi
                    for sub in range(2):
                        p0 = 64 * sub
                        r = 32 * ((2 * pi + sub) % 4)
                        bcp = ps1.tile([64, N], F32, tag="s1", name="bcp")
                        for n0 in (0, N // 2):
                            nc.tensor.matmul(bcp[:, n0:n0 + N // 2],
                                             onesr[r:r + 1, 0:64],
                                             rec4[r:r + 1, n0:n0 + N // 2],
                                             tile_position=(r, 0))
                        nc.vector.tensor_mul(out=oT[p0:p0 + D, hp, :],
                                             in0=bcp[:],
                                             in1=orws[pi][p0:p0 + D, :])

            # proj + double + spill x2
            for tt in range(NT):
                pr = ps2.tile([128, C], F32, tag="s2", name="pr")
                for k in range(CK):
                    for half, n0, nn in ((0, 0, 512), (1, 512, 256)):
                        nc.tensor.matmul(pr[:, n0:n0 + nn],
                                         oT[:, k, tt * 128:(tt + 1) * 128],
                                         wpT[:, k, n0:n0 + nn],
                                         start=(k == 0), stop=(k == CK - 1))
                x2a = xio.tile([128, C], F32, tag="x2s", name="x2a")
                nc.scalar.mul(out=x2a, in_=pr[:], mul=2.0)
                x2t = xio.tile([128, C], F32, tag="x2s", name="x2t")
                nc.gpsimd.tensor_add(out=x2t, in0=x2a, in1=pb2_bc)
                nc.scalar.dma_start(
                    out=x2d[t0 + tt * 128:t0 + (tt + 1) * 128, :], in_=x2t)

    # ================= stage B: fc1 =================
    with tc.tile_pool(name="wstage_b", bufs=2) as wstage_b, \
         tc.tile_pool(name="p2a", bufs=1) as p2a:
        wf1T = wbig.tile([128, CK, HID], F32R, tag="w")
        load_wT(io["fc1_w"], HID, C, wf1T, wstage_b)

        h2T = p2a.tile([128, CK, T], F32R, tag="h2T", name="h2T")
        for tt in range(T // 128):
            x2_t = xio.tile([128, C], F32, tag="xio", name="x2_t")
            nc.sync.dma_start(
                out=x2_t, in_=x2d[tt * 128:(tt + 1) * 128, :])
            h2 = layer_norm(x2_t, ln2w_bc, ln2b_bc, p2a)
            transpose_to(h2, h2T, tt)
        NQ = T // 512
        for j in range(JH):
            fps = [ps1.tile([128, 512], F32, tag="s1", name="fp")
                   for _ in range(NQ)]
            for k in range(CK):
                for q in range(NQ):
                    nc.tensor.matmul(fps[q][:],
                                     wf1T[:, k, j * 128:(j + 1) * 128],
                                     h2T[:, k, q * 512:(q + 1) * 512],
                                     start=(k == 0), stop=(k == CK - 1))
            for q in range(NQ):
                g = p2a.tile([128, 512], F32R, tag="gel", bufs=4, name="g")
                nc.scalar.activation(out=g, in_=fps[q][:], func=AF.Gelu,
                                     bias=fc1b_t[:, j:j + 1])
                nc.scalar.dma_start(out=f1d[j, :, q * 512:(q + 1) * 512],
                                    in_=g)

    # ================= stage C: fc2 + residual =================
    with tc.tile_pool(name="wstage_c", bufs=2) as wstage_c, \
         tc.tile_pool(name="p2b", bufs=1) as p2b:
        wf2T = wbig.tile([128, JH, C], F32R, tag="w")
        load_wT(io["fc2_w"], C, HID, wf2T, wstage_c)

        for ch in range(T // TC2):
            t0 = ch * TC2
            f1h = []
            for hf in range(2):
                f1t = p2b.tile([128, JH // 2, TC2], F32R, tag="f1in", bufs=3,
                               name="f1t")
                nc.sync.dma_start(
                    out=f1t,
                    in_=f1d[hf * (JH // 2):(hf + 1) * (JH // 2),
                            :, t0:t0 + TC2].rearrange("j p t -> p j t"))
                f1h.append(f1t)
            for tt in range(TC2 // 128):
                x2_t = xio.tile([128, C], F32, tag="xio", name="x2_t")
                nc.sync.dma_start(
                    out=x2_t, in_=x2d[t0 + tt * 128:t0 + (tt + 1) * 128, :])
                x2pb = xio.tile([128, C], F32, tag="xio", name="x2pb")
                nc.vector.tensor_add(out=x2pb, in0=x2_t, in1=fc2b_bc)
                f2 = ps2.tile([128, C], F32, tag="s2", name="f2")
                for k in range(JH):
                    for half, n0, nn in ((0, 0, 512), (1, 512, 256)):
                        nc.tensor.matmul(f2[:, n0:n0 + nn],
                                         f1h[k // (JH // 2)][
                                             :, k % (JH // 2),
                                             tt * 128:(tt + 1) * 128],
                                         wf2T[:, k, n0:n0 + nn],
                                         start=(k == 0), stop=(k == JH - 1))
                o_t = p2b.tile([128, C], F32, tag="outt", bufs=2, name="o_t")
                nc.vector.tensor_add(out=o_t, in0=f2[:], in1=x2pb)
                nc.sync.dma_start(
                    out=io["out"][t0 + tt * 128:t0 + (tt + 1) * 128, :], in_=o_t)


_CACHE = {}


def _act_table_override():
    """Drop the exp-only / ln-only ACT table sets so walrus selects
    natural_log_exp_and_others — the kernel alternates Exp and Ln per head
    and per-LN-tile, and each table switch costs ~1.5us on ScalarE."""
    return  # any act-root override breaks NEFF exec on the axon terminal
    try:
        from neuronxcc.driver.Job import Job
        from neuronxcc.driver.jobs.support.FindActInfo import findActInfoFile
        orig = findActInfoFile(Job.getPackageDir(), "gen3")
        d = json.load(open(orig))
        pref = [s for s in d["act_func_sets"]
                if s["name"] == "natural_log_exp_and_others"]
        rest = [s for s in d["act_func_sets"]
                if s["name"] != "natural_log_exp_and_others"]
        d["act_func_sets"] = pref + rest
        tmp = tempfile.mkdtemp(prefix="act_override_")
        src_dir = os.path.dirname(orig)
        base = os.path.basename(orig)
        for f in os.listdir(src_dir):
            if f != base:
                os.symlink(os.path.join(src_dir, f), os.path.join(tmp, f))
        path = os.path.join(tmp, base)
        with open(path, "w") as fh:
            json.dump(d, fh)
        os.environ["BASS_ACT_ROOT_JSON_PATH"] = path
    except Exception:
        pass


def _build():
    if "nc" in _CACHE:
        return _CACHE["nc"]
    _act_table_override()
    nc = bacc.Bacc("TRN2", target_bir_lowering=False, debug=False,
                   num_devices=NCORES)
    io = {}
    io["x"] = nc.dram_tensor("x", [T, C], F32, kind="ExternalInput").ap()
    for name, shape in [("ln1_w", [C]), ("ln1_b", [C]), ("qkv_w", [3 * C, C]),
                        ("proj_w", [C, C]), ("proj_b", [C]), ("ln2_w", [C]),
                        ("ln2_b", [C]), ("fc1_w", [HID, C]), ("fc1_b", [HID]),
                        ("fc2_w", [C, HID]), ("fc2_b", [C])]:
        io[name] = nc.dram_tensor(name, shape, F32, kind="ExternalInput").ap()
    io["out"] = nc.dram_tensor("out", [T, C], F32, kind="ExternalOutput").ap()

    with tile.TileContext(nc) as tc:
        with ExitStack() as ctx:
            _emit(tc, io, ctx)
    nc.compile()
    _CACHE["nc"] = nc
    return nc


def kernel(**inputs):
    nc = _build()
    arrs = {k: np.ascontiguousarray(np.asarray(v, dtype=np.float32))
            for k, v in inputs.items()}
    x = arrs.pop("x").reshape(B, N, C)
    in_maps = []
    for c in range(NCORES):
        m = dict(arrs)
        m["x"] = np.ascontiguousarray(x[c * BPC:(c + 1) * BPC].reshape(T, C))
        in_maps.append(m)
    res = run_bass_kernel_spmd(nc, in_maps, core_ids=list(range(NCORES)))
    out = np.concatenate(
        [r["out"].reshape(BPC, N, C) for r in res.results], axis=0)
    return out.astype(np.float32)


if __name__ == "__main__":
    rng = np.random.default_rng(0)
    ins = {
        "x": rng.standard_normal((B, N, C), dtype=np.float32),
        "ln1_w": np.ones(C, np.float32), "ln1_b": np.zeros(C, np.float32),
        "qkv_w": rng.standard_normal((3 * C, C), dtype=np.float32) / np.sqrt(C),
        "proj_w": rng.standard_normal((C, C), dtype=np.float32) / np.sqrt(C),
        "proj_b": np.zeros(C, np.float32),
        "ln2_w": np.ones(C, np.float32), "ln2_b": np.zeros(C, np.float32),
        "fc1_w": rng.standard_normal((HID, C), dtype=np.float32) / np.sqrt(C),
        "fc1_b": np.zeros(HID, np.float32),
        "fc2_w": rng.standard_normal((C, HID), dtype=np.float32) / np.sqrt(HID),
        "fc2_b": np.zeros(C, np.float32),
    }
    out = kernel(**ins)
    print("out", out.shape, out.dtype, np.abs(out).max())



# revision 17
# speedup vs baseline: 1.1659x; 1.1659x over previous
"""TRN2 Bass kernel: transformer Block (LN->MHA->2x residual->LN->MLP) for
B=32,N=512,C=768,H=12. Data-parallel over batch across 8 NeuronCores (4
items/core). All GEMMs in bf16 (1 cyc/row on PE, f32 PSUM accumulate),
which halves SBUF so the whole working set stays on-chip: no DRAM scratch
at all (x2 residual + fc1 activations live in SBUF).

Per-core schedule (engines balanced so ScalarE exp is the only
attention-phase serial resource and PE is the only MLP serial resource):
  prologue: qkv/proj weights DMA -> PE-transpose -> bf16 (proj pre-scaled
    by 2 to fold the residual doubling)
  A1 (per item): LN1 (stats->batched rsqrt->apply, LN affine folded into
    the h0T transpose evac) -> h0T -> qk matmuls -> v matmuls; all 4
    items' qk/v stay resident in bf16
  A2 (all items x 12 heads): scoresT = kT.T@qT into 2-strip PSUM tiles ->
    one exp per [128,1024] (no max-sub; scores are N(0,1)-scale) ->
    [v|1]-augmented AV matmul (oT + denominators in one pass) ->
    denominators gathered -> batched reciprocal -> PE-broadcast ->
    normalize into oT.  wf1 weight DMA streams during A2.
  A3 (per item): proj matmuls -> x2 = 2*proj+2*proj_b resident in SBUF
    bf16; wf1T PE-transposes interleaved; wf2 DMA streams.
  MLP: wf2T transposes + LN2 stats pass (batched rsqrt), then per
    512-token chunk: LN2 apply -> h2T -> fc1 -> gelu (bf16, SBUF) -> fc2
    -> +x2 +fc2_b -> out.  fc1 output is never spilled.
"""
import numpy as np
from contextlib import ExitStack

import concourse.bass as bass
import concourse.tile as tile
import concourse.bacc as bacc
from concourse import mybir
from concourse.bass_utils import run_bass_kernel_spmd
from concourse.masks import make_identity

F32 = mybir.dt.float32
F32R = mybir.dt.float32r
BF16 = mybir.dt.bfloat16
AF = mybir.ActivationFunctionType
ALU = mybir.AluOpType

B, N, C = 32, 512, 768
H, D = 12, 64
HID = 4 * C
EPS = 1e-5
NCORES = 8
BPC = B // NCORES            # batch items per core
T = BPC * N                  # tokens per core
CK = C // 128                # 6 contraction chunks over C
FQK = (2 * C) // 128         # 12 feature tiles for q+k
JH = HID // 128              # 24 hidden feature tiles
NT = N // 128                # 4 token tiles per item
SCALE = D ** -0.5
TC = 512                     # MLP token chunk
NCH = T // TC                # 4 chunks


def _bc(ap, p=128):
    """Broadcast a 1-D DRAM AP across p partitions (stride-0 partition dim)."""
    return bass.AP(tensor=ap.tensor, offset=ap.offset, ap=[[0, p]] + list(ap.ap))


def _emit(tc, io, ctx):
    nc = tc.nc

    consts = ctx.enter_context(tc.tile_pool(name="consts", bufs=1))
    wstage = ctx.enter_context(tc.tile_pool(name="wstage", bufs=2))
    arena = ctx.enter_context(tc.tile_pool(name="arena", bufs=1))

    # ---------------- constants ----------------
    ident32 = consts.tile([128, 128], F32)
    make_identity(nc, ident32)
    identb = consts.tile([128, 128], BF16)
    nc.vector.tensor_copy(out=identb, in_=ident32)
    onesf = consts.tile([128, 64], F32)
    nc.vector.memset(onesf, 1.0)
    onesr = consts.tile([128, 64], F32R)
    nc.vector.tensor_copy(out=onesr, in_=onesf)
    epst = consts.tile([128, 1], F32)
    nc.vector.memset(epst, EPS)

    w1c = consts.tile([128, CK], F32)
    nc.sync.dma_start(out=w1c, in_=io["ln1_w"].rearrange("(k p) -> p k", p=128))
    b1c = consts.tile([128, CK], F32)
    nc.sync.dma_start(out=b1c, in_=io["ln1_b"].rearrange("(k p) -> p k", p=128))
    w2c = consts.tile([128, CK], F32)
    nc.sync.dma_start(out=w2c, in_=io["ln2_w"].rearrange("(k p) -> p k", p=128))
    b2c = consts.tile([128, CK], F32)
    nc.sync.dma_start(out=b2c, in_=io["ln2_b"].rearrange("(k p) -> p k", p=128))
    pb2_bc = consts.tile([128, C], F32)
    nc.sync.dma_start(out=pb2_bc, in_=_bc(io["proj_b"]))
    nc.scalar.mul(out=pb2_bc, in_=pb2_bc, mul=2.0)
    fc2b_bc = consts.tile([128, C], F32)
    nc.sync.dma_start(out=fc2b_bc, in_=_bc(io["fc2_b"]))
    fc1b_t = consts.tile([128, JH], F32)
    nc.sync.dma_start(out=fc1b_t, in_=io["fc1_b"].rearrange("(j p) -> p j", p=128))

    # ---------------- weight transposition helpers ----------------
    evac_ctr = [0]

    def start_wT(w_ap, nrows, ncols):
        """Issue piece DMAs for w [nrows, ncols]; return piece list."""
        nj, nk = nrows // 128, ncols // 128
        wr = w_ap.rearrange("(j p) c -> p j c", p=128)
        pieces = []
        for j in range(nj):
            for c0 in range(0, nk, 6):
                cn = min(6, nk - c0)
                piece = wstage.tile([128, 768], F32, tag="wstage", name="piece")
                nc.sync.dma_start(out=piece[:, 0:cn * 128],
                                  in_=wr[:, j, c0 * 128:(c0 + cn) * 128])
                pieces.append((piece, j, c0, cn))
        return pieces

    def emit_wT(pieces, dst, ps, lo=0, hi=None, scale=None):
        """PE-transpose staged pieces into dst [128, nk, nrows] BF16."""
        for (piece, j, c0, cn) in pieces[lo:hi]:
            for k in range(cn):
                tp = ps.tile([128, 128], F32, tag="m", name="tp")
                nc.tensor.transpose(tp[:], piece[:, k * 128:(k + 1) * 128],
                                    ident32[:])
                dslice = dst[:, c0 + k, j * 128:(j + 1) * 128]
                if evac_ctr[0] % 2 == 0:
                    if scale is None:
                        nc.vector.tensor_copy(out=dslice, in_=tp[:])
                    else:
                        nc.vector.tensor_scalar_mul(out=dslice, in0=tp[:],
                                                    scalar1=scale)
                else:
                    if scale is None:
                        nc.scalar.copy(out=dslice, in_=tp[:])
                    else:
                        nc.scalar.activation(out=dslice, in_=tp[:],
                                             func=AF.Copy, scale=scale)
                evac_ctr[0] += 1

    # ================= stage A =================
    if True:
        wpT = arena.tile([128, CK, C], BF16, tag="wp")
        qk_sb = arena.tile([128, BPC, FQK, N], BF16, tag="t1")
        v_sb = arena.tile([128, BPC, NT, H, D + 1], BF16, tag="t2")
        for b in range(BPC):
            nc.vector.tensor_copy(
                out=v_sb[:, b, :, :, D:D + 1],
                in_=onesf[:, 0:NT * H].rearrange("p (a h c) -> p a h c",
                                                  a=NT, h=H))

        # ---- prologue + A1 ----
        with tc.tile_pool(name="a1t", bufs=1) as a1t, \
             tc.tile_pool(name="psA1", bufs=4, space="PSUM") as psA1:

            # first item's x tiles ahead of the weight pieces on the queue
            xts = {}
            for b in range(1):
                for tt in range(NT):
                    x_t = a1t.tile([128, C], F32, tag="xt", bufs=4, name="x_t")
                    nc.sync.dma_start(
                        out=x_t, in_=io["x"][b * N + tt * 128:
                                             b * N + (tt + 1) * 128, :])
                    xts[(b, tt)] = x_t

            wqkvT = arena.tile([128, CK, 3 * C], BF16, tag="t3")
            qkv_pieces = start_wT(io["qkv_w"], 3 * C, C)
            wp_pieces = start_wT(io["proj_w"], C, C)
            emit_wT(qkv_pieces, wqkvT, psA1)
            emit_wT(wp_pieces, wpT, psA1, scale=2.0)

            for b in range(BPC):
                # prefetch next item's x
                if b + 1 < BPC:
                    for tt in range(NT):
                        x_t = a1t.tile([128, C], F32, tag="xt", bufs=4,
                                       name="x_t")
                        nc.sync.dma_start(
                            out=x_t, in_=io["x"][(b + 1) * N + tt * 128:
                                                 (b + 1) * N + (tt + 1) * 128, :])
                        xts[(b + 1, tt)] = x_t

                # LN1 stats -> batched rsqrt -> apply; affine folded in evac
                mv4 = a1t.tile([128, NT, nc.vector.BN_AGGR_DIM], F32,
                               tag="mv4", bufs=2, name="mv4")
                for tt in range(NT):
                    x_t = xts.pop((b, tt))
                    st = a1t.tile([128, 3, nc.vector.BN_STATS_DIM], F32,
                                  tag="bnst", bufs=2, name="st")
                    for i in range(3):
                        nc.vector.bn_stats(out=st[:, i, :],
                                           in_=x_t[:, 256 * i:256 * (i + 1)])
                    nc.vector.bn_aggr(out=mv4[:, tt, :], in_=st)
                    xts[(b, tt)] = x_t
                rstd4 = a1t.tile([128, NT], F32, tag="rstd", bufs=2,
                                 name="rstd4")
                nc.scalar.activation(out=rstd4, in_=mv4[:, :, 1:2], func=AF.Sqrt,
                                     bias=epst)
                nc.vector.reciprocal(out=rstd4, in_=rstd4)

                h0T = a1t.tile([128, CK, N], BF16, tag="h0T", bufs=2,
                               name="h0T")
                for tt in range(NT):
                    x_t = xts.pop((b, tt))
                    h = a1t.tile([128, C], BF16, tag="h", bufs=1, name="h")
                    nc.vector.tensor_scalar(out=h, in0=x_t,
                                            scalar1=mv4[:, tt, 0:1],
                                            scalar2=rstd4[:, tt:tt + 1],
                                            op0=ALU.subtract, op1=ALU.mult)
                    for k in range(CK):
                        tp = psA1.tile([128, 128], BF16, tag="m", name="tp")
                        nc.tensor.transpose(tp[:], h[:, k * 128:(k + 1) * 128],
                                            identb[:])
                        dslice = h0T[:, k, tt * 128:(tt + 1) * 128]
                        if k % 2 == 0:
                            nc.vector.tensor_scalar(
                                out=dslice, in0=tp[:], scalar1=w1c[:, k:k + 1],
                                scalar2=b1c[:, k:k + 1], op0=ALU.mult,
                                op1=ALU.add)
                        else:
                            nc.scalar.activation(
                                out=dslice, in_=tp[:], func=AF.Identity,
                                scale=w1c[:, k:k + 1], bias=b1c[:, k:k + 1])

                # q,k: feature tile j holds heads 2j / 2j+1 stacked
                for j in range(FQK):
                    qp = psA1.tile([128, N], F32, tag="m", name="qp")
                    for k in range(CK):
                        nc.tensor.matmul(qp[:],
                                         wqkvT[:, k, j * 128:(j + 1) * 128],
                                         h0T[:, k, :], start=(k == 0),
                                         stop=(k == CK - 1))
                    if j % 2 == 0:
                        nc.vector.tensor_copy(out=qk_sb[:, b, j, :], in_=qp[:])
                    else:
                        nc.scalar.copy(out=qk_sb[:, b, j, :], in_=qp[:])

                # v (tokens on partitions)
                for tt in range(NT):
                    vp = psA1.tile([128, C], F32, tag="w", bufs=2, name="vp")
                    for k in range(CK):
                        for n0, nn in ((0, 512), (512, 256)):
                            nc.tensor.matmul(vp[:, n0:n0 + nn],
                                             h0T[:, k, tt * 128:(tt + 1) * 128],
                                             wqkvT[:, k, 2 * C + n0:
                                                   2 * C + n0 + nn],
                                             start=(k == 0), stop=(k == CK - 1))
                    vdst = v_sb[:, b, tt, :, 0:D]
                    vsrc = vp.rearrange("p (h d) -> p h d", h=H)
                    if tt % 2 == 0:
                        nc.vector.tensor_copy(out=vdst, in_=vsrc)
                    else:
                        nc.scalar.copy(out=vdst, in_=vsrc)

        # ---- A2: attention for all items; wf1 DMA streams ----
        oTs = []
        with tc.tile_pool(name="a2t", bufs=1) as a2t, \
             tc.tile_pool(name="psA2", bufs=1, space="PSUM") as psA2:

            wf1T = arena.tile([128, CK, HID], BF16, tag="t5")
            f1_pieces = start_wT(io["fc1_w"], HID, C)

            for b in range(BPC):
                oT = arena.tile([128, CK, N], BF16, tag="t4", bufs=BPC,
                                 name="oT")
                oTs.append(oT)
                for q4 in range(H // 4):
                    srow = a2t.tile([128, N], F32, tag="srow", bufs=2,
                                    name="srow")
                    orws = []
                    for pi in range(2):
                        hp = 2 * q4 + pi
                        kj = FQK // 2 + hp
                        for sub in range(2):
                            h = 2 * hp + sub
                            p0 = 64 * sub
                            r = 32 * (2 * pi + sub)
                            av = psA2.tile([D + 1, N], F32, tag="av", bufs=2,
                                           name="av")
                            sc4 = psA2.tile([128, NT, N], F32, tag="sc",
                                            bufs=1, name="sc4")
                            for c in range(NT):
                                nc.tensor.matmul(
                                    sc4[:, c, :],
                                    qk_sb[p0:p0 + D, b, kj,
                                          c * 128:(c + 1) * 128],
                                    qk_sb[p0:p0 + D, b, hp, :])
                            ex4 = a2t.tile([128, NT, N], BF16, tag="ex",
                                           bufs=2, name="ex4")
                            nc.scalar.activation(out=ex4, in_=sc4[:, :, :],
                                                 func=AF.Exp, scale=SCALE)
                            for c in range(NT):
                                nc.tensor.matmul(av[:], v_sb[:, b, c, h, :],
                                                 ex4[:, c, :],
                                                 start=(c == 0),
                                                 stop=(c == NT - 1))
                            nc.vector.tensor_copy(out=srow[r:r + 1, :],
                                                  in_=av[D:D + 1, :])
                            orw = a2t.tile([64, N], BF16, tag="orw", bufs=4,
                                           name="orw")
                            if (2 * pi + sub) % 3 == 2:
                                nc.scalar.copy(out=orw, in_=av[0:D, :])
                            else:
                                nc.vector.tensor_copy(out=orw, in_=av[0:D, :])
                            orws.append(orw)
                    rec4 = a2t.tile([128, N], F32R, tag="rec", bufs=2,
                                    name="rec4")
                    with nc.allow_low_precision(reason="softmax denom recip"):
                        nc.vector.reciprocal(out=rec4[0:97, 0:N // 2],
                                             in_=srow[0:97, 0:N // 2])
                        nc.vector.reciprocal(out=rec4[0:97, N // 2:N],
                                             in_=srow[0:97, N // 2:N])
                    for pi in range(2):
                        hp = 2 * q4 + pi
                        for sub in range(2):
                            p0 = 64 * sub
                            idx = 2 * pi + sub
                            r = 32 * idx
                            bcp = psA2.tile([64, N], F32, tag="bc", bufs=2,
                                            name="bcp")
                            for n0 in (0, N // 2):
                                nc.tensor.matmul(bcp[:, n0:n0 + N // 2],
                                                 onesr[r:r + 1, 0:64],
                                                 rec4[r:r + 1, n0:n0 + N // 2],
                                                 tile_position=(r, 0))
                            nc.vector.tensor_mul(out=oT[p0:p0 + D, hp, :],
                                                 in0=bcp[:],
                                                 in1=orws[idx])

    # ---- A3: proj + x2; wf1T transposes interleaved; wf2 DMA ----
        x2_sb = arena.tile([128, BPC, NT, C], BF16, tag="t3")
        with tc.tile_pool(name="psA3", bufs=4, space="PSUM") as psA3:
            f2_pieces = start_wT(io["fc2_w"], C, HID)
            npz = len(f1_pieces)
            for b in range(BPC):
                emit_wT(f1_pieces, wf1T, psA3,
                        lo=b * npz // BPC, hi=(b + 1) * npz // BPC)
                oT = oTs[b]
                for tt in range(NT):
                    pr = psA3.tile([128, C], F32, tag="w", bufs=2, name="pr")
                    for k in range(CK):
                        for n0, nn in ((0, 512), (512, 256)):
                            nc.tensor.matmul(pr[:, n0:n0 + nn],
                                             oT[:, k, tt * 128:(tt + 1) * 128],
                                             wpT[:, k, n0:n0 + nn],
                                             start=(k == 0), stop=(k == CK - 1))
                    nc.vector.tensor_add(out=x2_sb[:, b, tt, :], in0=pr[:],
                                         in1=pb2_bc)

    # ================= MLP (fused fc1 -> gelu -> fc2, no spills) =========
    with tc.tile_pool(name="mlp", bufs=1) as mlp, \
         tc.tile_pool(name="psM", bufs=4, space="PSUM") as psM:

        wf2T = arena.tile([128, JH, C], BF16, tag="t1")

        # LN2 stats for all tiles (DVE) while wf2T transposes run (PE)
        mv16 = mlp.tile([128, T // 128, nc.vector.BN_AGGR_DIM], F32,
                        name="mv16")
        for tl in range(T // 128):
            b, tt = tl // NT, tl % NT
            st = mlp.tile([128, 3, nc.vector.BN_STATS_DIM], F32,
                          tag="bnst", bufs=2, name="st")
            for i in range(3):
                nc.vector.bn_stats(out=st[:, i, :],
                                   in_=x2_sb[:, b, tt, 256 * i:256 * (i + 1)])
            nc.vector.bn_aggr(out=mv16[:, tl, :], in_=st)
        rstd16 = mlp.tile([128, T // 128], F32, name="rstd16")
        nc.scalar.activation(out=rstd16, in_=mv16[:, :, 1:2], func=AF.Sqrt,
                             bias=epst)
        nc.vector.reciprocal(out=rstd16, in_=rstd16)
        emit_wT(f2_pieces, wf2T, psM)

        g = arena.tile([128, JH, TC], BF16, tag="t2", name="g")
        for ch in range(NCH):
            # LN2 apply + transpose into h2T
            h2T = arena.tile([128, CK, TC], BF16, tag="t4", bufs=BPC,
                             name="h2T")
            for ti in range(TC // 128):
                tl = ch * (TC // 128) + ti
                b, tt = tl // NT, tl % NT
                h2 = mlp.tile([128, C], BF16, tag="h2", bufs=2, name="h2")
                nc.vector.tensor_scalar(out=h2, in0=x2_sb[:, b, tt, :],
                                        scalar1=mv16[:, tl, 0:1],
                                        scalar2=rstd16[:, tl:tl + 1],
                                        op0=ALU.subtract, op1=ALU.mult)
                for k in range(CK):
                    tp = psM.tile([128, 128], BF16, tag="m", name="tp")
                    nc.tensor.transpose(tp[:], h2[:, k * 128:(k + 1) * 128],
                                        identb[:])
                    dslice = h2T[:, k, ti * 128:(ti + 1) * 128]
                    if k % 2 == 0:
                        nc.vector.tensor_scalar(
                            out=dslice, in0=tp[:], scalar1=w2c[:, k:k + 1],
                            scalar2=b2c[:, k:k + 1], op0=ALU.mult, op1=ALU.add)
                    else:
                        nc.scalar.activation(
                            out=dslice, in_=tp[:], func=AF.Identity,
                            scale=w2c[:, k:k + 1], bias=b2c[:, k:k + 1])

            # fc1 + gelu
            for j in range(JH):
                fp = psM.tile([128, TC], F32, tag="m", name="fp")
                for k in range(CK):
                    nc.tensor.matmul(fp[:], wf1T[:, k, j * 128:(j + 1) * 128],
                                     h2T[:, k, :], start=(k == 0),
                                     stop=(k == CK - 1))
                nc.scalar.activation(out=g[:, j, :], in_=fp[:], func=AF.Gelu,
                                     bias=fc1b_t[:, j:j + 1])

            # fc2 + residual + out
            for ti in range(TC // 128):
                tl = ch * (TC // 128) + ti
                b, tt = tl // NT, tl % NT
                f2 = psM.tile([128, C], F32, tag="w", bufs=2, name="f2")
                for k in range(JH):
                    for n0, nn in ((0, 512), (512, 256)):
                        nc.tensor.matmul(f2[:, n0:n0 + nn],
                                         g[:, k, ti * 128:(ti + 1) * 128],
                                         wf2T[:, k, n0:n0 + nn],
                                         start=(k == 0), stop=(k == JH - 1))
                s1 = mlp.tile([128, C], F32, tag="s1", bufs=2, name="s1")
                nc.vector.tensor_add(out=s1, in0=f2[:],
                                     in1=x2_sb[:, b, tt, :])
                o_t = mlp.tile([128, C], F32, tag="ot", bufs=2, name="o_t")
                nc.gpsimd.tensor_add(out=o_t, in0=s1, in1=fc2b_bc)
                nc.scalar.dma_start(
                    out=io["out"][tl * 128:(tl + 1) * 128, :], in_=o_t)


_CACHE = {}


def _build():
    if "nc" in _CACHE:
        return _CACHE["nc"]
    nc = bacc.Bacc("TRN2", target_bir_lowering=False, debug=False,
                   num_devices=NCORES)
    io = {}
    io["x"] = nc.dram_tensor("x", [T, C], F32, kind="ExternalInput").ap()
    for name, shape in [("ln1_w", [C]), ("ln1_b", [C]), ("qkv_w", [3 * C, C]),
                        ("proj_w", [C, C]), ("proj_b", [C]), ("ln2_w", [C]),
                        ("ln2_b", [C]), ("fc1_w", [HID, C]), ("fc1_b", [HID]),
                        ("fc2_w", [C, HID]), ("fc2_b", [C])]:
        io[name] = nc.dram_tensor(name, shape, F32, kind="ExternalInput").ap()
    io["out"] = nc.dram_tensor("out", [T, C], F32, kind="ExternalOutput").ap()

    with tile.TileContext(nc) as tc:
        with ExitStack() as ctx:
            _emit(tc, io, ctx)
    nc.compile()
    _CACHE["nc"] = nc
    return nc


def kernel(**inputs):
    nc = _build()
    arrs = {k: np.ascontiguousarray(np.asarray(v, dtype=np.float32))
            for k, v in inputs.items()}
    x = arrs.pop("x").reshape(B, N, C)
    in_maps = []
    for c in range(NCORES):
        m = dict(arrs)
        m["x"] = np.ascontiguousarray(x[c * BPC:(c + 1) * BPC].reshape(T, C))
        in_maps.append(m)
    res = run_bass_kernel_spmd(nc, in_maps, core_ids=list(range(NCORES)))
    out = np.concatenate(
        [r["out"].reshape(BPC, N, C) for r in res.results], axis=0)
    return out.astype(np.float32)


if __name__ == "__main__":
    rng = np.random.default_rng(0)
    ins = {
        "x": rng.standard_normal((B, N, C), dtype=np.float32),
        "ln1_w": np.ones(C, np.float32), "ln1_b": np.zeros(C, np.float32),
        "qkv_w": rng.standard_normal((3 * C, C), dtype=np.float32) / np.sqrt(C),
        "proj_w": rng.standard_normal((C, C), dtype=np.float32) / np.sqrt(C),
        "proj_b": np.zeros(C, np.float32),
        "ln2_w": np.ones(C, np.float32), "ln2_b": np.zeros(C, np.float32),
        "fc1_w": rng.standard_normal((HID, C), dtype=np.float32) / np.sqrt(C),
        "fc1_b": np.zeros(HID, np.float32),
        "fc2_w": rng.standard_normal((C, HID), dtype=np.float32) / np.sqrt(HID),
        "fc2_b": np.zeros(C, np.float32),
    }
    out = kernel(**ins)
    print("out", out.shape, out.dtype, np.abs(out).max())


# revision 19
# speedup vs baseline: 1.2032x; 1.0321x over previous
"""TRN2 Bass kernel: transformer Block (LN->MHA->2x residual->LN->MLP) for
B=32,N=512,C=768,H=12. Data-parallel over batch across 8 NeuronCores (4
items/core). All GEMMs in bf16 (1 cyc/row on PE, f32 PSUM accumulate),
which halves SBUF so the whole working set stays on-chip: no DRAM scratch
at all (x2 residual + fc1 activations live in SBUF).

Per-core schedule (engines balanced so ScalarE exp is the only
attention-phase serial resource and PE is the only MLP serial resource):
  prologue: qkv/proj weights DMA -> PE-transpose -> bf16 (proj pre-scaled
    by 2 to fold the residual doubling)
  A1 (per item): LN1 (stats->batched rsqrt->apply, LN affine folded into
    the h0T transpose evac) -> h0T -> qk matmuls -> v matmuls; all 4
    items' qk/v stay resident in bf16
  A2 (all items x 12 heads): scoresT = kT.T@qT into 2-strip PSUM tiles ->
    one exp per [128,1024] (no max-sub; scores are N(0,1)-scale) ->
    [v|1]-augmented AV matmul (oT + denominators in one pass) ->
    denominators gathered -> batched reciprocal -> PE-broadcast ->
    normalize into oT.  wf1 weight DMA streams during A2.
  A3 (per item): proj matmuls -> x2 = 2*proj+2*proj_b resident in SBUF
    bf16; wf1T PE-transposes interleaved; wf2 DMA streams.
  MLP: wf2T transposes + LN2 stats pass (batched rsqrt), then per
    512-token chunk: LN2 apply -> h2T -> fc1 -> gelu (bf16, SBUF) -> fc2
    -> +x2 +fc2_b -> out.  fc1 output is never spilled.
"""
import numpy as np
from contextlib import ExitStack

import concourse.bass as bass
import concourse.tile as tile
import concourse.bacc as bacc
from concourse import mybir
from concourse.bass_utils import run_bass_kernel_spmd
from concourse.masks import make_identity

F32 = mybir.dt.float32
F32R = mybir.dt.float32r
BF16 = mybir.dt.bfloat16
AF = mybir.ActivationFunctionType
ALU = mybir.AluOpType

B, N, C = 32, 512, 768
H, D = 12, 64
HID = 4 * C
EPS = 1e-5
NCORES = 8
BPC = B // NCORES            # batch items per core
T = BPC * N                  # tokens per core
CK = C // 128                # 6 contraction chunks over C
FQK = (2 * C) // 128         # 12 feature tiles for q+k
JH = HID // 128              # 24 hidden feature tiles
NT = N // 128                # 4 token tiles per item
SCALE = D ** -0.5
TC = 512                     # MLP token chunk
NCH = T // TC                # 4 chunks


def _bc(ap, p=128):
    """Broadcast a 1-D DRAM AP across p partitions (stride-0 partition dim)."""
    return bass.AP(tensor=ap.tensor, offset=ap.offset, ap=[[0, p]] + list(ap.ap))


def _emit(tc, io, ctx):
    nc = tc.nc

    consts = ctx.enter_context(tc.tile_pool(name="consts", bufs=1))
    wstage = ctx.enter_context(tc.tile_pool(name="wstage", bufs=2))
    arena = ctx.enter_context(tc.tile_pool(name="arena", bufs=1))

    # ---------------- constants ----------------
    ident32 = consts.tile([128, 128], F32)
    make_identity(nc, ident32)
    identb = consts.tile([128, 128], BF16)
    nc.vector.tensor_copy(out=identb, in_=ident32)
    onesf = consts.tile([128, 64], F32)
    nc.vector.memset(onesf, 1.0)
    onesr = consts.tile([128, 64], F32R)
    nc.vector.tensor_copy(out=onesr, in_=onesf)
    epst = consts.tile([128, 1], F32)
    nc.vector.memset(epst, EPS)

    w1c = consts.tile([128, CK], F32)
    nc.sync.dma_start(out=w1c, in_=io["ln1_w"].rearrange("(k p) -> p k", p=128))
    b1c = consts.tile([128, CK], F32)
    nc.sync.dma_start(out=b1c, in_=io["ln1_b"].rearrange("(k p) -> p k", p=128))
    w2c = consts.tile([128, CK], F32)
    nc.sync.dma_start(out=w2c, in_=io["ln2_w"].rearrange("(k p) -> p k", p=128))
    b2c = consts.tile([128, CK], F32)
    nc.sync.dma_start(out=b2c, in_=io["ln2_b"].rearrange("(k p) -> p k", p=128))
    pb2_bc = consts.tile([128, C], F32)
    nc.sync.dma_start(out=pb2_bc, in_=_bc(io["proj_b"]))
    nc.scalar.mul(out=pb2_bc, in_=pb2_bc, mul=2.0)
    fc2b_bc = consts.tile([128, C], F32)
    nc.sync.dma_start(out=fc2b_bc, in_=_bc(io["fc2_b"]))
    fc1b_t = consts.tile([128, JH], F32)
    nc.sync.dma_start(out=fc1b_t, in_=io["fc1_b"].rearrange("(j p) -> p j", p=128))

    # ---------------- weight transposition helpers ----------------
    evac_ctr = [0]

    def start_wT(w_ap, nrows, ncols):
        """Issue piece DMAs for w [nrows, ncols]; return piece list."""
        nj, nk = nrows // 128, ncols // 128
        wr = w_ap.rearrange("(j p) c -> p j c", p=128)
        pieces = []
        for j in range(nj):
            for c0 in range(0, nk, 6):
                cn = min(6, nk - c0)
                piece = wstage.tile([128, 768], F32, tag="wstage", name="piece")
                nc.sync.dma_start(out=piece[:, 0:cn * 128],
                                  in_=wr[:, j, c0 * 128:(c0 + cn) * 128])
                pieces.append((piece, j, c0, cn))
        return pieces

    def emit_wT(pieces, dst, ps, lo=0, hi=None, scale=None):
        """PE-transpose staged pieces into dst [128, nk, nrows] BF16."""
        for (piece, j, c0, cn) in pieces[lo:hi]:
            for k in range(cn):
                tp = ps.tile([128, 128], F32, tag="m", name="tp")
                nc.tensor.transpose(tp[:], piece[:, k * 128:(k + 1) * 128],
                                    ident32[:])
                dslice = dst[:, c0 + k, j * 128:(j + 1) * 128]
                if evac_ctr[0] % 2 == 0:
                    if scale is None:
                        nc.vector.tensor_copy(out=dslice, in_=tp[:])
                    else:
                        nc.vector.tensor_scalar_mul(out=dslice, in0=tp[:],
                                                    scalar1=scale)
                else:
                    if scale is None:
                        nc.scalar.copy(out=dslice, in_=tp[:])
                    else:
                        nc.scalar.activation(out=dslice, in_=tp[:],
                                             func=AF.Copy, scale=scale)
                evac_ctr[0] += 1

    # ================= stage A =================
    if True:
        wpT = arena.tile([128, CK, C], BF16, tag="wp")
        qk_sb = arena.tile([128, BPC, FQK, N], BF16, tag="t1")
        v_sb = arena.tile([128, BPC, NT, H, D + 1], BF16, tag="t2")
        for b in range(BPC):
            nc.vector.tensor_copy(
                out=v_sb[:, b, :, :, D:D + 1],
                in_=onesf[:, 0:NT * H].rearrange("p (a h c) -> p a h c",
                                                  a=NT, h=H))

        # ---- prologue + A1 ----
        with tc.tile_pool(name="a1t", bufs=1) as a1t, \
             tc.tile_pool(name="psA1", bufs=4, space="PSUM") as psA1:

            # first item's x tiles ahead of the weight pieces on the queue
            xts = {}
            for b in range(1):
                for tt in range(NT):
                    x_t = a1t.tile([128, C], F32, tag="xt", bufs=4, name="x_t")
                    nc.sync.dma_start(
                        out=x_t, in_=io["x"][b * N + tt * 128:
                                             b * N + (tt + 1) * 128, :])
                    xts[(b, tt)] = x_t

            wqkvT = arena.tile([128, CK, 3 * C], BF16, tag="t3")
            qkv_pieces = start_wT(io["qkv_w"], 3 * C, C)
            wp_pieces = start_wT(io["proj_w"], C, C)
            emit_wT(qkv_pieces, wqkvT, psA1)
            emit_wT(wp_pieces, wpT, psA1, scale=2.0)

            for b in range(BPC):
                # prefetch next item's x
                if b + 1 < BPC:
                    for tt in range(NT):
                        x_t = a1t.tile([128, C], F32, tag="xt", bufs=4,
                                       name="x_t")
                        nc.sync.dma_start(
                            out=x_t, in_=io["x"][(b + 1) * N + tt * 128:
                                                 (b + 1) * N + (tt + 1) * 128, :])
                        xts[(b + 1, tt)] = x_t

                # LN1 stats -> batched rsqrt -> apply; affine folded in evac
                mv4 = a1t.tile([128, NT, nc.vector.BN_AGGR_DIM], F32,
                               tag="mv4", bufs=2, name="mv4")
                for tt in range(NT):
                    x_t = xts.pop((b, tt))
                    st = a1t.tile([128, 3, nc.vector.BN_STATS_DIM], F32,
                                  tag="bnst", bufs=2, name="st")
                    for i in range(3):
                        nc.vector.bn_stats(out=st[:, i, :],
                                           in_=x_t[:, 256 * i:256 * (i + 1)])
                    nc.vector.bn_aggr(out=mv4[:, tt, :], in_=st)
                    xts[(b, tt)] = x_t
                rstd4 = a1t.tile([128, NT], F32, tag="rstd", bufs=2,
                                 name="rstd4")
                nc.scalar.activation(out=rstd4, in_=mv4[:, :, 1:2], func=AF.Sqrt,
                                     bias=epst)
                nc.vector.reciprocal(out=rstd4, in_=rstd4)

                h0T = a1t.tile([128, CK, N], BF16, tag="h0T", bufs=2,
                               name="h0T")
                for tt in range(NT):
                    x_t = xts.pop((b, tt))
                    h = a1t.tile([128, C], BF16, tag="h", bufs=1, name="h")
                    nc.vector.tensor_scalar(out=h, in0=x_t,
                                            scalar1=mv4[:, tt, 0:1],
                                            scalar2=rstd4[:, tt:tt + 1],
                                            op0=ALU.subtract, op1=ALU.mult)
                    for k in range(CK):
                        tp = psA1.tile([128, 128], BF16, tag="m", name="tp")
                        nc.tensor.transpose(tp[:], h[:, k * 128:(k + 1) * 128],
                                            identb[:])
                        dslice = h0T[:, k, tt * 128:(tt + 1) * 128]
                        if k % 2 == 0:
                            nc.vector.tensor_scalar(
                                out=dslice, in0=tp[:], scalar1=w1c[:, k:k + 1],
                                scalar2=b1c[:, k:k + 1], op0=ALU.mult,
                                op1=ALU.add)
                        else:
                            nc.scalar.activation(
                                out=dslice, in_=tp[:], func=AF.Identity,
                                scale=w1c[:, k:k + 1], bias=b1c[:, k:k + 1])

                # q,k: feature tile j holds heads 2j / 2j+1 stacked
                for j in range(FQK):
                    qp = psA1.tile([128, N], F32, tag="m", name="qp")
                    for k in range(CK):
                        nc.tensor.matmul(qp[:],
                                         wqkvT[:, k, j * 128:(j + 1) * 128],
                                         h0T[:, k, :], start=(k == 0),
                                         stop=(k == CK - 1))
                    if j % 2 == 0:
                        nc.vector.tensor_copy(out=qk_sb[:, b, j, :], in_=qp[:])
                    else:
                        nc.scalar.copy(out=qk_sb[:, b, j, :], in_=qp[:])

                # v (tokens on partitions)
                for tt in range(NT):
                    vp = psA1.tile([128, C], F32, tag="w", bufs=2, name="vp")
                    for k in range(CK):
                        for n0, nn in ((0, 512), (512, 256)):
                            nc.tensor.matmul(vp[:, n0:n0 + nn],
                                             h0T[:, k, tt * 128:(tt + 1) * 128],
                                             wqkvT[:, k, 2 * C + n0:
                                                   2 * C + n0 + nn],
                                             start=(k == 0), stop=(k == CK - 1))
                    vdst = v_sb[:, b, tt, :, 0:D]
                    vsrc = vp.rearrange("p (h d) -> p h d", h=H)
                    if tt % 2 == 0:
                        nc.vector.tensor_copy(out=vdst, in_=vsrc)
                    else:
                        nc.scalar.copy(out=vdst, in_=vsrc)

        # ---- A2: attention for all items; wf1 DMA streams ----
        oTs = []
        with tc.tile_pool(name="a2t", bufs=1) as a2t, \
             tc.tile_pool(name="psA2", bufs=1, space="PSUM") as psA2:

            wf1T = arena.tile([128, CK, HID], BF16, tag="t5")
            f1_pieces = start_wT(io["fc1_w"], HID, C)

            for b in range(BPC):
                oT = arena.tile([128, CK, N], BF16, tag="t4", bufs=BPC,
                                 name="oT")
                oTs.append(oT)
                for q4 in range(H // 4):
                    srow = a2t.tile([128, N], F32, tag="srow", bufs=2,
                                    name="srow")
                    orws = []
                    for pi in range(2):
                        hp = 2 * q4 + pi
                        kj = FQK // 2 + hp
                        for sub in range(2):
                            h = 2 * hp + sub
                            p0 = 64 * sub
                            r = 32 * (2 * pi + sub)
                            av = psA2.tile([D + 1, N], F32, tag="av", bufs=2,
                                           name="av")
                            exs = []
                            for half in range(2):
                                sc2 = psA2.tile([128, 2, N], F32, tag="sc",
                                                bufs=2, name="sc2")
                                for ci in range(2):
                                    c = 2 * half + ci
                                    nc.tensor.matmul(
                                        sc2[:, ci, :],
                                        qk_sb[p0:p0 + D, b, kj,
                                              c * 128:(c + 1) * 128],
                                        qk_sb[p0:p0 + D, b, hp, :])
                                ex2 = a2t.tile([128, 2, N], BF16, tag="ex",
                                               bufs=4, name="ex2")
                                nc.scalar.activation(out=ex2, in_=sc2[:, :, :],
                                                     func=AF.Exp, scale=SCALE)
                                exs.append(ex2)
                            for c in range(NT):
                                nc.tensor.matmul(av[:], v_sb[:, b, c, h, :],
                                                 exs[c // 2][:, c % 2, :],
                                                 start=(c == 0),
                                                 stop=(c == NT - 1))
                            nc.vector.tensor_copy(out=srow[r:r + 1, :],
                                                  in_=av[D:D + 1, :])
                            orw = a2t.tile([64, N], BF16, tag="orw", bufs=4,
                                           name="orw")
                            if (2 * pi + sub) % 3 == 2:
                                nc.scalar.copy(out=orw, in_=av[0:D, :])
                            else:
                                nc.vector.tensor_copy(out=orw, in_=av[0:D, :])
                            orws.append(orw)
                    recf = a2t.tile([128, N], F32, tag="recf", bufs=2,
                                    name="recf")
                    nc.vector.reciprocal_approx_fast(out=recf[0:97, :],
                                                     in_=srow[0:97, :])
                    rec4 = a2t.tile([128, N], F32R, tag="rec", bufs=2,
                                    name="rec4")
                    with nc.allow_low_precision(reason="softmax denom recip"):
                        nc.vector.tensor_copy(out=rec4[0:97, :],
                                              in_=recf[0:97, :])
                    for pi in range(2):
                        hp = 2 * q4 + pi
                        for sub in range(2):
                            p0 = 64 * sub
                            idx = 2 * pi + sub
                            r = 32 * idx
                            bcp = psA2.tile([64, N], F32, tag="bc", bufs=2,
                                            name="bcp")
                            for n0 in (0, N // 2):
                                nc.tensor.matmul(bcp[:, n0:n0 + N // 2],
                                                 onesr[r:r + 1, 0:64],
                                                 rec4[r:r + 1, n0:n0 + N // 2],
                                                 tile_position=(r, 0))
                            nc.vector.tensor_mul(out=oT[p0:p0 + D, hp, :],
                                                 in0=bcp[:],
                                                 in1=orws[idx])

    # ---- A3: proj + x2; wf1T transposes interleaved; wf2 DMA ----
        x2_sb = arena.tile([128, BPC, NT, C], BF16, tag="t3")
        with tc.tile_pool(name="psA3", bufs=4, space="PSUM") as psA3:
            f2_pieces = start_wT(io["fc2_w"], C, HID)
            npz = len(f1_pieces)
            for b in range(BPC):
                emit_wT(f1_pieces, wf1T, psA3,
                        lo=b * npz // BPC, hi=(b + 1) * npz // BPC)
                oT = oTs[b]
                for tt in range(NT):
                    pr = psA3.tile([128, C], F32, tag="w", bufs=2, name="pr")
                    for k in range(CK):
                        for n0, nn in ((0, 512), (512, 256)):
                            nc.tensor.matmul(pr[:, n0:n0 + nn],
                                             oT[:, k, tt * 128:(tt + 1) * 128],
                                             wpT[:, k, n0:n0 + nn],
                                             start=(k == 0), stop=(k == CK - 1))
                    nc.vector.tensor_add(out=x2_sb[:, b, tt, :], in0=pr[:],
                                         in1=pb2_bc)

    # ================= MLP (fused fc1 -> gelu -> fc2, no spills) =========
    with tc.tile_pool(name="mlp", bufs=1) as mlp, \
         tc.tile_pool(name="psM", bufs=4, space="PSUM") as psM:

        wf2T = arena.tile([128, JH, C], BF16, tag="t1")

        # LN2 stats for all tiles (DVE) while wf2T transposes run (PE)
        mv16 = mlp.tile([128, T // 128, nc.vector.BN_AGGR_DIM], F32,
                        name="mv16")
        for tl in range(T // 128):
            b, tt = tl // NT, tl % NT
            st = mlp.tile([128, 3, nc.vector.BN_STATS_DIM], F32,
                          tag="bnst", bufs=2, name="st")
            for i in range(3):
                nc.vector.bn_stats(out=st[:, i, :],
                                   in_=x2_sb[:, b, tt, 256 * i:256 * (i + 1)])
            nc.vector.bn_aggr(out=mv16[:, tl, :], in_=st)
        rstd16 = mlp.tile([128, T // 128], F32, name="rstd16")
        nc.scalar.activation(out=rstd16, in_=mv16[:, :, 1:2], func=AF.Sqrt,
                             bias=epst)
        nc.vector.reciprocal(out=rstd16, in_=rstd16)
        emit_wT(f2_pieces, wf2T, psM)

        g = arena.tile([128, JH, TC], BF16, tag="t2", name="g")
        for ch in range(NCH):
            # LN2 apply + transpose into h2T
            h2T = arena.tile([128, CK, TC], BF16, tag="t4", bufs=BPC,
                             name="h2T")
            for ti in range(TC // 128):
                tl = ch * (TC // 128) + ti
                b, tt = tl // NT, tl % NT
                h2 = mlp.tile([128, C], BF16, tag="h2", bufs=2, name="h2")
                nc.vector.tensor_scalar(out=h2, in0=x2_sb[:, b, tt, :],
                                        scalar1=mv16[:, tl, 0:1],
                                        scalar2=rstd16[:, tl:tl + 1],
                                        op0=ALU.subtract, op1=ALU.mult)
                for k in range(CK):
                    tp = psM.tile([128, 128], BF16, tag="m", name="tp")
                    nc.tensor.transpose(tp[:], h2[:, k * 128:(k + 1) * 128],
                                        identb[:])
                    dslice = h2T[:, k, ti * 128:(ti + 1) * 128]
                    if k % 2 == 0:
                        nc.vector.tensor_scalar(
                            out=dslice, in0=tp[:], scalar1=w2c[:, k:k + 1],
                            scalar2=b2c[:, k:k + 1], op0=ALU.mult, op1=ALU.add)
                    else:
                        nc.scalar.activation(
                            out=dslice, in_=tp[:], func=AF.Identity,
                            scale=w2c[:, k:k + 1], bias=b2c[:, k:k + 1])

            # fc1 + gelu
            for j in range(JH):
                fp = psM.tile([128, TC], F32, tag="m", name="fp")
                for k in range(CK):
                    nc.tensor.matmul(fp[:], wf1T[:, k, j * 128:(j + 1) * 128],
                                     h2T[:, k, :], start=(k == 0),
                                     stop=(k == CK - 1))
                nc.scalar.activation(out=g[:, j, :], in_=fp[:], func=AF.Gelu,
                                     bias=fc1b_t[:, j:j + 1])

            # fc2 + residual + out
            for ti in range(TC // 128):
                tl = ch * (TC // 128) + ti
                b, tt = tl // NT, tl % NT
                f2 = psM.tile([128, C], F32, tag="w", bufs=2, name="f2")
                for k in range(JH):
                    for n0, nn in ((0, 512), (512, 256)):
                        nc.tensor.matmul(f2[:, n0:n0 + nn],
                                         g[:, k, ti * 128:(ti + 1) * 128],
                                         wf2T[:, k, n0:n0 + nn],
                                         start=(k == 0), stop=(k == JH - 1))
                s1 = mlp.tile([128, C], F32, tag="s1", bufs=2, name="s1")
                nc.vector.tensor_add(out=s1, in0=f2[:],
                                     in1=x2_sb[:, b, tt, :])
                o_t = mlp.tile([128, C], F32, tag="ot", bufs=2, name="o_t")
                nc.gpsimd.tensor_add(out=o_t, in0=s1, in1=fc2b_bc)
                nc.scalar.dma_start(
                    out=io["out"][tl * 128:(tl + 1) * 128, :], in_=o_t)


_CACHE = {}


def _build():
    if "nc" in _CACHE:
        return _CACHE["nc"]
    nc = bacc.Bacc("TRN2", target_bir_lowering=False, debug=False,
                   num_devices=NCORES)
    io = {}
    io["x"] = nc.dram_tensor("x", [T, C], F32, kind="ExternalInput").ap()
    for name, shape in [("ln1_w", [C]), ("ln1_b", [C]), ("qkv_w", [3 * C, C]),
                        ("proj_w", [C, C]), ("proj_b", [C]), ("ln2_w", [C]),
                        ("ln2_b", [C]), ("fc1_w", [HID, C]), ("fc1_b", [HID]),
                        ("fc2_w", [C, HID]), ("fc2_b", [C])]:
        io[name] = nc.dram_tensor(name, shape, F32, kind="ExternalInput").ap()
    io["out"] = nc.dram_tensor("out", [T, C], F32, kind="ExternalOutput").ap()

    with tile.TileContext(nc) as tc:
        with ExitStack() as ctx:
            _emit(tc, io, ctx)
    nc.compile()
    _CACHE["nc"] = nc
    return nc


def kernel(**inputs):
    nc = _build()
    arrs = {k: np.ascontiguousarray(np.asarray(v, dtype=np.float32))
            for k, v in inputs.items()}
    x = arrs.pop("x").reshape(B, N, C)
    in_maps = []
    for c in range(NCORES):
        m = dict(arrs)
        m["x"] = np.ascontiguousarray(x[c * BPC:(c + 1) * BPC].reshape(T, C))
        in_maps.append(m)
    res = run_bass_kernel_spmd(nc, in_maps, core_ids=list(range(NCORES)))
    out = np.concatenate(
        [r["out"].reshape(BPC, N, C) for r in res.results], axis=0)
    return out.astype(np.float32)


if __name__ == "__main__":
    rng = np.random.default_rng(0)
    ins = {
        "x": rng.standard_normal((B, N, C), dtype=np.float32),
        "ln1_w": np.ones(C, np.float32), "ln1_b": np.zeros(C, np.float32),
        "qkv_w": rng.standard_normal((3 * C, C), dtype=np.float32) / np.sqrt(C),
        "proj_w": rng.standard_normal((C, C), dtype=np.float32) / np.sqrt(C),
        "proj_b": np.zeros(C, np.float32),
        "ln2_w": np.ones(C, np.float32), "ln2_b": np.zeros(C, np.float32),
        "fc1_w": rng.standard_normal((HID, C), dtype=np.float32) / np.sqrt(C),
        "fc1_b": np.zeros(HID, np.float32),
        "fc2_w": rng.standard_normal((C, HID), dtype=np.float32) / np.sqrt(HID),
        "fc2_b": np.zeros(C, np.float32),
    }
    out = kernel(**ins)
    print("out", out.shape, out.dtype, np.abs(out).max())


# revision 22
# speedup vs baseline: 1.3307x; 1.1059x over previous
"""TRN2 Bass kernel: transformer Block (LN->MHA->2x residual->LN->MLP) for
B=32,N=512,C=768,H=12. Data-parallel over batch across 8 NeuronCores (4
items/core). All GEMMs in bf16 (1 cyc/row on PE, f32 PSUM accumulate),
which halves SBUF so the whole working set stays on-chip: no DRAM scratch
at all (x2 residual + fc1 activations live in SBUF).

Per-core schedule (engines balanced so ScalarE exp is the only
attention-phase serial resource and PE is the only MLP serial resource):
  prologue: qkv/proj weights DMA -> PE-transpose -> bf16 (proj pre-scaled
    by 2 to fold the residual doubling)
  A1 (per item): LN1 (stats->batched rsqrt->apply, LN affine folded into
    the h0T transpose evac) -> h0T -> qk matmuls -> v matmuls; all 4
    items' qk/v stay resident in bf16
  A2 (all items x 12 heads): scoresT = kT.T@qT into 2-strip PSUM tiles ->
    one exp per [128,1024] (no max-sub; scores are N(0,1)-scale) ->
    [v|1]-augmented AV matmul (oT + denominators in one pass) ->
    denominators gathered -> batched reciprocal -> PE-broadcast ->
    normalize into oT.  wf1 weight DMA streams during A2.
  A3 (per item): proj matmuls -> x2 = 2*proj+2*proj_b resident in SBUF
    bf16; wf1T PE-transposes interleaved; wf2 DMA streams.
  MLP: wf2T transposes + LN2 stats pass (batched rsqrt), then per
    512-token chunk: LN2 apply -> h2T -> fc1 -> gelu (bf16, SBUF) -> fc2
    -> +x2 +fc2_b -> out.  fc1 output is never spilled.
"""
import numpy as np
from contextlib import ExitStack

import concourse.bass as bass
import concourse.tile as tile
import concourse.bacc as bacc
from concourse import mybir
from concourse.bass_utils import run_bass_kernel_spmd
from concourse.masks import make_identity

F32 = mybir.dt.float32
F32R = mybir.dt.float32r
BF16 = mybir.dt.bfloat16
AF = mybir.ActivationFunctionType
ALU = mybir.AluOpType

B, N, C = 32, 512, 768
H, D = 12, 64
HID = 4 * C
EPS = 1e-5
NCORES = 8
BPC = B // NCORES            # batch items per core
T = BPC * N                  # tokens per core
CK = C // 128                # 6 contraction chunks over C
FQK = (2 * C) // 128         # 12 feature tiles for q+k
JH = HID // 128              # 24 hidden feature tiles
NT = N // 128                # 4 token tiles per item
SCALE = D ** -0.5
TC = 512                     # MLP token chunk
NCH = T // TC                # 4 chunks


def _bc(ap, p=128):
    """Broadcast a 1-D DRAM AP across p partitions (stride-0 partition dim)."""
    return bass.AP(tensor=ap.tensor, offset=ap.offset, ap=[[0, p]] + list(ap.ap))


def _emit(tc, io, ctx):
    nc = tc.nc

    consts = ctx.enter_context(tc.tile_pool(name="consts", bufs=1))
    wstage = ctx.enter_context(tc.tile_pool(name="wstage", bufs=2))
    arena = ctx.enter_context(tc.tile_pool(name="arena", bufs=1))

    # ---------------- constants ----------------
    ident32 = consts.tile([128, 128], F32)
    make_identity(nc, ident32)
    identb = consts.tile([128, 128], BF16)
    nc.vector.tensor_copy(out=identb, in_=ident32)
    onesf = consts.tile([128, 64], F32)
    nc.vector.memset(onesf, 1.0)
    onesr = consts.tile([128, 64], F32R)
    nc.vector.tensor_copy(out=onesr, in_=onesf)
    epst = consts.tile([128, 1], F32)
    nc.vector.memset(epst, EPS)

    w1c = consts.tile([128, CK], F32)
    nc.sync.dma_start(out=w1c, in_=io["ln1_w"].rearrange("(k p) -> p k", p=128))
    b1c = consts.tile([128, CK], F32)
    nc.sync.dma_start(out=b1c, in_=io["ln1_b"].rearrange("(k p) -> p k", p=128))
    w2c = consts.tile([128, CK], F32)
    nc.sync.dma_start(out=w2c, in_=io["ln2_w"].rearrange("(k p) -> p k", p=128))
    b2c = consts.tile([128, CK], F32)
    nc.sync.dma_start(out=b2c, in_=io["ln2_b"].rearrange("(k p) -> p k", p=128))
    pb2_bc = consts.tile([128, C], F32)
    nc.sync.dma_start(out=pb2_bc, in_=_bc(io["proj_b"]))
    nc.scalar.mul(out=pb2_bc, in_=pb2_bc, mul=2.0)
    fc2b_bc = consts.tile([128, C], F32)
    nc.sync.dma_start(out=fc2b_bc, in_=_bc(io["fc2_b"]))
    fc1b_t = consts.tile([128, JH], F32)
    nc.sync.dma_start(out=fc1b_t, in_=io["fc1_b"].rearrange("(j p) -> p j", p=128))

    # ---------------- weight transposition helpers ----------------
    evac_ctr = [0]

    def start_wT(w_ap, nrows, ncols):
        """Issue casting piece DMAs (f32 DRAM -> bf16 SBUF via SWDGE)."""
        nj, nk = nrows // 128, ncols // 128
        wr = w_ap.rearrange("(j p) c -> p j c", p=128)
        pieces = []
        for j in range(nj):
            for c0 in range(0, nk, 6):
                cn = min(6, nk - c0)
                piece = wstage.tile([128, 768], BF16, tag="wstage",
                                    name="piece")
                nc.gpsimd.dma_start(out=piece[:, 0:cn * 128],
                                    in_=wr[:, j, c0 * 128:(c0 + cn) * 128])
                pieces.append((piece, j, c0, cn))
        return pieces

    def emit_wT(pieces, dst, ps, lo=0, hi=None, scale=None):
        """PE-transpose staged pieces into dst [128, nk, nrows] BF16."""
        for (piece, j, c0, cn) in pieces[lo:hi]:
            for k in range(cn):
                tp = ps.tile([128, 128], BF16, tag="m", name="tp")
                nc.tensor.transpose(tp[:], piece[:, k * 128:(k + 1) * 128],
                                    identb[:])
                dslice = dst[:, c0 + k, j * 128:(j + 1) * 128]
                if evac_ctr[0] % 2 == 0:
                    if scale is None:
                        nc.vector.tensor_copy(out=dslice, in_=tp[:])
                    else:
                        nc.vector.tensor_scalar_mul(out=dslice, in0=tp[:],
                                                    scalar1=scale)
                else:
                    if scale is None:
                        nc.scalar.copy(out=dslice, in_=tp[:])
                    else:
                        nc.scalar.activation(out=dslice, in_=tp[:],
                                             func=AF.Copy, scale=scale)
                evac_ctr[0] += 1

    # ================= stage A =================
    if True:
        wpT = arena.tile([128, CK, C], BF16, tag="wp")
        qk_sb = arena.tile([128, BPC, FQK, N], BF16, tag="t1")
        v_sb = arena.tile([128, BPC, NT, H, D + 1], BF16, tag="t2")
        for b in range(BPC):
            nc.vector.tensor_copy(
                out=v_sb[:, b, :, :, D:D + 1],
                in_=onesf[:, 0:NT * H].rearrange("p (a h c) -> p a h c",
                                                  a=NT, h=H))

        # ---- prologue + A1 ----
        with tc.tile_pool(name="a1t", bufs=1) as a1t, \
             tc.tile_pool(name="psA1", bufs=4, space="PSUM") as psA1:

            # first item's x tiles ahead of the weight pieces on the queue
            xts = {}
            for b in range(1):
                for tt in range(NT):
                    x_t = a1t.tile([128, C], F32, tag="xt", bufs=4, name="x_t")
                    nc.sync.dma_start(
                        out=x_t, in_=io["x"][b * N + tt * 128:
                                             b * N + (tt + 1) * 128, :])
                    xts[(b, tt)] = x_t

            wqkvT = arena.tile([128, CK, 3 * C], BF16, tag="t3")
            qkv_pieces = start_wT(io["qkv_w"], 3 * C, C)
            wp_pieces = start_wT(io["proj_w"], C, C)
            emit_wT(qkv_pieces, wqkvT, psA1)
            emit_wT(wp_pieces, wpT, psA1, scale=2.0)

            for b in range(BPC):
                # prefetch next item's x
                if b + 1 < BPC:
                    for tt in range(NT):
                        x_t = a1t.tile([128, C], F32, tag="xt", bufs=4,
                                       name="x_t")
                        nc.sync.dma_start(
                            out=x_t, in_=io["x"][(b + 1) * N + tt * 128:
                                                 (b + 1) * N + (tt + 1) * 128, :])
                        xts[(b + 1, tt)] = x_t

                # LN1 stats -> batched rsqrt -> apply; affine folded in evac
                mv4 = a1t.tile([128, NT, nc.vector.BN_AGGR_DIM], F32,
                               tag="mv4", bufs=2, name="mv4")
                for tt in range(NT):
                    x_t = xts.pop((b, tt))
                    st = a1t.tile([128, 3, nc.vector.BN_STATS_DIM], F32,
                                  tag="bnst", bufs=2, name="st")
                    for i in range(3):
                        nc.vector.bn_stats(out=st[:, i, :],
                                           in_=x_t[:, 256 * i:256 * (i + 1)])
                    nc.vector.bn_aggr(out=mv4[:, tt, :], in_=st)
                    xts[(b, tt)] = x_t
                rstd4 = a1t.tile([128, NT], F32, tag="rstd", bufs=2,
                                 name="rstd4")
                nc.scalar.activation(out=rstd4, in_=mv4[:, :, 1:2], func=AF.Sqrt,
                                     bias=epst)
                nc.vector.reciprocal(out=rstd4, in_=rstd4)

                h0T = a1t.tile([128, CK, N], BF16, tag="h0T", bufs=2,
                               name="h0T")
                for tt in range(NT):
                    x_t = xts.pop((b, tt))
                    h = a1t.tile([128, C], BF16, tag="h", bufs=1, name="h")
                    nc.vector.tensor_scalar(out=h, in0=x_t,
                                            scalar1=mv4[:, tt, 0:1],
                                            scalar2=rstd4[:, tt:tt + 1],
                                            op0=ALU.subtract, op1=ALU.mult)
                    for k in range(CK):
                        tp = psA1.tile([128, 128], BF16, tag="m", name="tp")
                        nc.tensor.transpose(tp[:], h[:, k * 128:(k + 1) * 128],
                                            identb[:])
                        dslice = h0T[:, k, tt * 128:(tt + 1) * 128]
                        if k % 2 == 0:
                            nc.vector.tensor_scalar(
                                out=dslice, in0=tp[:], scalar1=w1c[:, k:k + 1],
                                scalar2=b1c[:, k:k + 1], op0=ALU.mult,
                                op1=ALU.add)
                        else:
                            nc.scalar.activation(
                                out=dslice, in_=tp[:], func=AF.Identity,
                                scale=w1c[:, k:k + 1], bias=b1c[:, k:k + 1])

                # q,k: feature tile j holds heads 2j / 2j+1 stacked
                for j in range(FQK):
                    qp = psA1.tile([128, N], F32, tag="m", name="qp")
                    for k in range(CK):
                        nc.tensor.matmul(qp[:],
                                         wqkvT[:, k, j * 128:(j + 1) * 128],
                                         h0T[:, k, :], start=(k == 0),
                                         stop=(k == CK - 1))
                    if j % 2 == 0:
                        nc.vector.tensor_copy(out=qk_sb[:, b, j, :], in_=qp[:])
                    else:
                        nc.scalar.copy(out=qk_sb[:, b, j, :], in_=qp[:])

                # v (tokens on partitions)
                for tt in range(NT):
                    vp = psA1.tile([128, C], F32, tag="w", bufs=2, name="vp")
                    for k in range(CK):
                        for n0, nn in ((0, 512), (512, 256)):
                            nc.tensor.matmul(vp[:, n0:n0 + nn],
                                             h0T[:, k, tt * 128:(tt + 1) * 128],
                                             wqkvT[:, k, 2 * C + n0:
                                                   2 * C + n0 + nn],
                                             start=(k == 0), stop=(k == CK - 1))
                    vdst = v_sb[:, b, tt, :, 0:D]
                    vsrc = vp.rearrange("p (h d) -> p h d", h=H)
                    if tt % 2 == 0:
                        nc.vector.tensor_copy(out=vdst, in_=vsrc)
                    else:
                        nc.scalar.copy(out=vdst, in_=vsrc)

        # ---- A2: attention for all items; wf1 DMA streams ----
        oTs = []
        with tc.tile_pool(name="a2t", bufs=1) as a2t, \
             tc.tile_pool(name="psA2", bufs=1, space="PSUM") as psA2:

            wf1T = arena.tile([128, CK, HID], BF16, tag="t5")
            f1_pieces = start_wT(io["fc1_w"], HID, C)

            # software-pipelined: scores+exp of unit u+1 issue before the
            # AV/extract retirement of unit u, so ScalarE exps run
            # back-to-back and the PE never waits on an exp.
            groups = {}

            def retire(unit):
                b, q4, pi, sub, exs = unit
                h = 2 * (2 * q4 + pi) + sub
                idx = 2 * pi + sub
                r = 32 * idx
                g = groups[(b, q4)]
                av = psA2.tile([D + 1, N], F32, tag="av", bufs=2, name="av")
                for c in range(NT):
                    nc.tensor.matmul(av[:], v_sb[:, b, c, h, :],
                                     exs[c // 2][:, c % 2, :],
                                     start=(c == 0), stop=(c == NT - 1))
                nc.vector.tensor_copy(out=g["srow"][r:r + 1, :],
                                      in_=av[D:D + 1, :])
                orw = a2t.tile([64, N], BF16, tag="orw", bufs=8, name="orw")
                if idx % 3 == 2:
                    nc.scalar.copy(out=orw, in_=av[0:D, :])
                else:
                    nc.vector.tensor_copy(out=orw, in_=av[0:D, :])
                g["orws"].append(orw)
                if idx == 3:
                    finish_group(b, q4)

            def finish_group(b, q4):
                g = groups.pop((b, q4))
                srow = g["srow"]
                recf = a2t.tile([128, N], F32, tag="recf", bufs=1,
                                name="recf")
                nc.vector.reciprocal_approx_fast(out=recf[0:97, :],
                                                 in_=srow[0:97, :])
                rec4 = a2t.tile([128, N], F32R, tag="rec", bufs=1,
                                name="rec4")
                with nc.allow_low_precision(reason="softmax denom recip"):
                    nc.vector.tensor_copy(out=rec4[0:97, :],
                                          in_=recf[0:97, :])
                for pi in range(2):
                    hp = 2 * q4 + pi
                    for sub in range(2):
                        p0 = 64 * sub
                        idx = 2 * pi + sub
                        r = 32 * idx
                        bcp = psA2.tile([64, N], F32, tag="bc", bufs=2,
                                        name="bcp")
                        for n0 in (0, N // 2):
                            nc.tensor.matmul(bcp[:, n0:n0 + N // 2],
                                             onesr[r:r + 1, 0:64],
                                             rec4[r:r + 1, n0:n0 + N // 2],
                                             tile_position=(r, 0))
                        nc.vector.tensor_mul(out=g["oT"][p0:p0 + D, hp, :],
                                             in0=bcp[:],
                                             in1=g["orws"][idx])

            pend = None
            for b in range(BPC):
                oT = arena.tile([128, CK, N], BF16, tag="t4", bufs=BPC,
                                name="oT")
                oTs.append(oT)
                for q4 in range(H // 4):
                    srow = a2t.tile([128, N], F32, tag="srow", bufs=2,
                                    name="srow")
                    groups[(b, q4)] = {"srow": srow, "orws": [], "oT": oT}
                    for pi in range(2):
                        hp = 2 * q4 + pi
                        kj = FQK // 2 + hp
                        for sub in range(2):
                            p0 = 64 * sub
                            exs = []
                            for half in range(2):
                                sc2 = psA2.tile([128, 2, N], F32, tag="sc",
                                                bufs=2, name="sc2")
                                for ci in range(2):
                                    c = 2 * half + ci
                                    nc.tensor.matmul(
                                        sc2[:, ci, :],
                                        qk_sb[p0:p0 + D, b, kj,
                                              c * 128:(c + 1) * 128],
                                        qk_sb[p0:p0 + D, b, hp, :])
                                ex2 = a2t.tile([128, 2, N], BF16, tag="ex",
                                               bufs=4, name="ex2")
                                nc.scalar.activation(out=ex2, in_=sc2[:, :, :],
                                                     func=AF.Exp, scale=SCALE)
                                exs.append(ex2)
                            if pend is not None:
                                retire(pend)
                            pend = (b, q4, pi, sub, exs)
            retire(pend)

    # ---- A3: proj + x2; wf1T transposes interleaved; wf2 DMA ----
        x2_sb = arena.tile([128, BPC, NT, C], BF16, tag="t3")
        with tc.tile_pool(name="psA3", bufs=4, space="PSUM") as psA3:
            f2_pieces = start_wT(io["fc2_w"], C, HID)
            npz = len(f1_pieces)
            for b in range(BPC):
                emit_wT(f1_pieces, wf1T, psA3,
                        lo=b * npz // BPC, hi=(b + 1) * npz // BPC)
                oT = oTs[b]
                for tt in range(NT):
                    pr = psA3.tile([128, C], F32, tag="w", bufs=2, name="pr")
                    for k in range(CK):
                        for n0, nn in ((0, 512), (512, 256)):
                            nc.tensor.matmul(pr[:, n0:n0 + nn],
                                             oT[:, k, tt * 128:(tt + 1) * 128],
                                             wpT[:, k, n0:n0 + nn],
                                             start=(k == 0), stop=(k == CK - 1))
                    nc.vector.tensor_add(out=x2_sb[:, b, tt, :], in0=pr[:],
                                         in1=pb2_bc)

    # ================= MLP (fused fc1 -> gelu -> fc2, no spills) =========
    with tc.tile_pool(name="mlp", bufs=1) as mlp, \
         tc.tile_pool(name="psM", bufs=4, space="PSUM") as psM:

        wf2T = arena.tile([128, JH, C], BF16, tag="t1")

        # LN2 stats for all tiles (DVE) while wf2T transposes run (PE)
        mv16 = mlp.tile([128, T // 128, nc.vector.BN_AGGR_DIM], F32,
                        name="mv16")
        for tl in range(T // 128):
            b, tt = tl // NT, tl % NT
            st = mlp.tile([128, 3, nc.vector.BN_STATS_DIM], F32,
                          tag="bnst", bufs=2, name="st")
            for i in range(3):
                nc.vector.bn_stats(out=st[:, i, :],
                                   in_=x2_sb[:, b, tt, 256 * i:256 * (i + 1)])
            nc.vector.bn_aggr(out=mv16[:, tl, :], in_=st)
        rstd16 = mlp.tile([128, T // 128], F32, name="rstd16")
        nc.scalar.activation(out=rstd16, in_=mv16[:, :, 1:2], func=AF.Sqrt,
                             bias=epst)
        nc.vector.reciprocal(out=rstd16, in_=rstd16)
        emit_wT(f2_pieces, wf2T, psM)

        g = arena.tile([128, JH, TC], BF16, tag="t2", name="g")
        for ch in range(NCH):
            # LN2 apply + transpose into h2T
            h2T = arena.tile([128, CK, TC], BF16, tag="t4", bufs=BPC,
                             name="h2T")
            for ti in range(TC // 128):
                tl = ch * (TC // 128) + ti
                b, tt = tl // NT, tl % NT
                h2 = mlp.tile([128, C], BF16, tag="h2", bufs=2, name="h2")
                nc.vector.tensor_scalar(out=h2, in0=x2_sb[:, b, tt, :],
                                        scalar1=mv16[:, tl, 0:1],
                                        scalar2=rstd16[:, tl:tl + 1],
                                        op0=ALU.subtract, op1=ALU.mult)
                for k in range(CK):
                    tp = psM.tile([128, 128], BF16, tag="m", name="tp")
                    nc.tensor.transpose(tp[:], h2[:, k * 128:(k + 1) * 128],
                                        identb[:])
                    dslice = h2T[:, k, ti * 128:(ti + 1) * 128]
                    if k % 2 == 0:
                        nc.vector.tensor_scalar(
                            out=dslice, in0=tp[:], scalar1=w2c[:, k:k + 1],
                            scalar2=b2c[:, k:k + 1], op0=ALU.mult, op1=ALU.add)
                    else:
                        nc.scalar.activation(
                            out=dslice, in_=tp[:], func=AF.Identity,
                            scale=w2c[:, k:k + 1], bias=b2c[:, k:k + 1])

            # fc1 + gelu
            for j in range(JH):
                fp = psM.tile([128, TC], F32, tag="m", name="fp")
                for k in range(CK):
                    nc.tensor.matmul(fp[:], wf1T[:, k, j * 128:(j + 1) * 128],
                                     h2T[:, k, :], start=(k == 0),
                                     stop=(k == CK - 1))
                nc.scalar.activation(out=g[:, j, :], in_=fp[:], func=AF.Gelu,
                                     bias=fc1b_t[:, j:j + 1])

            # fc2 + residual + out
            for ti in range(TC // 128):
                tl = ch * (TC // 128) + ti
                b, tt = tl // NT, tl % NT
                f2 = psM.tile([128, C], F32, tag="w", bufs=2, name="f2")
                for k in range(JH):
                    for n0, nn in ((0, 512), (512, 256)):
                        nc.tensor.matmul(f2[:, n0:n0 + nn],
                                         g[:, k, ti * 128:(ti + 1) * 128],
                                         wf2T[:, k, n0:n0 + nn],
                                         start=(k == 0), stop=(k == JH - 1))
                s1 = mlp.tile([128, C], F32, tag="s1", bufs=2, name="s1")
                nc.vector.tensor_add(out=s1, in0=f2[:],
                                     in1=x2_sb[:, b, tt, :])
                o_t = mlp.tile([128, C], F32, tag="ot", bufs=2, name="o_t")
                nc.gpsimd.tensor_add(out=o_t, in0=s1, in1=fc2b_bc)
                nc.scalar.dma_start(
                    out=io["out"][tl * 128:(tl + 1) * 128, :], in_=o_t)


_CACHE = {}


def _build():
    if "nc" in _CACHE:
        return _CACHE["nc"]
    nc = bacc.Bacc("TRN2", target_bir_lowering=False, debug=False,
                   num_devices=NCORES)
    io = {}
    io["x"] = nc.dram_tensor("x", [T, C], F32, kind="ExternalInput").ap()
    for name, shape in [("ln1_w", [C]), ("ln1_b", [C]), ("qkv_w", [3 * C, C]),
                        ("proj_w", [C, C]), ("proj_b", [C]), ("ln2_w", [C]),
                        ("ln2_b", [C]), ("fc1_w", [HID, C]), ("fc1_b", [HID]),
                        ("fc2_w", [C, HID]), ("fc2_b", [C])]:
        io[name] = nc.dram_tensor(name, shape, F32, kind="ExternalInput").ap()
    io["out"] = nc.dram_tensor("out", [T, C], F32, kind="ExternalOutput").ap()

    with tile.TileContext(nc) as tc:
        with ExitStack() as ctx:
            _emit(tc, io, ctx)
    nc.compile()
    _CACHE["nc"] = nc
    return nc


def kernel(**inputs):
    nc = _build()
    arrs = {k: np.ascontiguousarray(np.asarray(v, dtype=np.float32))
            for k, v in inputs.items()}
    x = arrs.pop("x").reshape(B, N, C)
    in_maps = []
    for c in range(NCORES):
        m = dict(arrs)
        m["x"] = np.ascontiguousarray(x[c * BPC:(c + 1) * BPC].reshape(T, C))
        in_maps.append(m)
    res = run_bass_kernel_spmd(nc, in_maps, core_ids=list(range(NCORES)))
    out = np.concatenate(
        [r["out"].reshape(BPC, N, C) for r in res.results], axis=0)
    return out.astype(np.float32)


if __name__ == "__main__":
    rng = np.random.default_rng(0)
    ins = {
        "x": rng.standard_normal((B, N, C), dtype=np.float32),
        "ln1_w": np.ones(C, np.float32), "ln1_b": np.zeros(C, np.float32),
        "qkv_w": rng.standard_normal((3 * C, C), dtype=np.float32) / np.sqrt(C),
        "proj_w": rng.standard_normal((C, C), dtype=np.float32) / np.sqrt(C),
        "proj_b": np.zeros(C, np.float32),
        "ln2_w": np.ones(C, np.float32), "ln2_b": np.zeros(C, np.float32),
        "fc1_w": rng.standard_normal((HID, C), dtype=np.float32) / np.sqrt(C),
        "fc1_b": np.zeros(HID, np.float32),
        "fc2_w": rng.standard_normal((C, HID), dtype=np.float32) / np.sqrt(HID),
        "fc2_b": np.zeros(C, np.float32),
    }
    out = kernel(**ins)
    print("out", out.shape, out.dtype, np.abs(out).max())
